# revision 7
# baseline (speedup 1.0000x reference)
"""Embedding lookup (out[b,s,:] = W[x[b,s],:] + b) on 8 Trainium2 NeuronCores.

Strategy: data-parallel over tokens. Each core receives the full W in its HBM
plus a 1/8 slice of the flattened ids, gathers its 1024 rows from W via
indirect DMA (int32 row offsets, one id per SBUF partition per instruction --
multi-id offset APs are mis-unrolled by the HW ucode), and stores a
[1024, 1024] output slice. The host concatenates the 8 slices. No
collectives, no masking: every id is in range on every core.

Raw Bass (no Tile): a two-engine pipeline. gpsimd issues the indirect
gathers (SWDGE, HBM->SBUF); sync issues the stores (HWDGE, SBUF->HBM),
each store chasing its gather via one semaphore. b is zero by this
problem's input spec; an exact host-side fallback handles nonzero b.

Per-core HBM traffic = 4 MiB gather-read + 4 MiB store-write, which is the
memory roofline for this op.
"""

import os
import numpy as np

try:
    from concourse import bass, mybir
    from concourse.bass_utils import run_bass_kernel_spmd
except ImportError:  # toolchain not on sys.path in a fresh dir
    import sys

    sys.path.insert(0, "/opt/trn_rl_repo")
    from concourse import bass, mybir
    from concourse.bass_utils import run_bass_kernel_spmd

N_CORES = 8
B, S = 4, 2048
V, D = 50304, 1024
P = 128
TOK = B * S  # 8192 tokens total
TPC = TOK // N_CORES  # 1024 tokens per core
NCHUNK = TPC // P  # 8 chunks of 128 tokens; chunk m holds tokens m*P + p

# Filled by kernel() when profiling is enabled (trace=True).
LAST_EXEC_NS = None
LAST_RESULTS = None


def build_nc(tpc=TPC, v=V, d=D):
    """One-core program; SPMD-identical across cores (inputs differ)."""
    nchunk = tpc // P
    assert nchunk * P == tpc
    nc = bass.Bass()
    ids = nc.declare_dram_parameter("ids", [P, nchunk], mybir.dt.int32, isOutput=False)
    W = nc.declare_dram_parameter("W", [v, d], mybir.dt.float32, isOutput=False)
    out = nc.declare_dram_parameter("out", [tpc, d], mybir.dt.float32, isOutput=True)

    import contextlib

    with contextlib.ExitStack() as ctx:
        ids_all = ctx.enter_context(
            nc.sbuf_tensor("ids_all", [P, nchunk], mybir.dt.int32)
        )
        g = ctx.enter_context(
            nc.sbuf_tensor("g", [P, nchunk * d], mybir.dt.float32)
        )
        ids_sem = ctx.enter_context(nc.semaphore("ids_sem"))
        s_sem = ctx.enter_context(nc.semaphore("s_sem"))
        # Concurrent DMAs on one queue complete out of order, so each gather
        # gets its own completion semaphore for its store to chase.
        g_sems = [
            ctx.enter_context(nc.semaphore(f"g_sem{m}")) for m in range(nchunk)
        ]
        block = ctx.enter_context(nc.Block())

        @block.gpsimd
        def _(gpsimd):
            gpsimd.wait_ge(ids_sem, 16)
            for m in range(nchunk):
                gpsimd.indirect_dma_start(
                    out=g[:, m * d : (m + 1) * d],
                    out_offset=None,
                    in_=W[:, :],
                    in_offset=bass.IndirectOffsetOnAxis(
                        ap=ids_all[:, m : m + 1], axis=0
                    ),
                ).then_inc(g_sems[m], 16)

        @block.sync
        def _(sync):
            sync.dma_start(out=ids_all[:], in_=ids[:, :]).then_inc(ids_sem, 16)
            for m in range(nchunk):
                sync.wait_ge(g_sems[m], 16)
                # chunk m: partition p holds row of token m*P + p
                sync.dma_start(
                    out=out[m * P : (m + 1) * P, :],
                    in_=g[:, m * d : (m + 1) * d],
                ).then_inc(s_sem, 16)
            sync.wait_ge(s_sem, 16 * nchunk)

    return nc


_NC_CACHE = {}


def _get_nc():
    if "nc" not in _NC_CACHE:
        _NC_CACHE["nc"] = build_nc()
    return _NC_CACHE["nc"]


def shard_ids(x):
    """[B,S] int32 -> per-core [P, NCHUNK] id grids; ids[p, m] = id of token m*P+p."""
    flat = np.ascontiguousarray(x).reshape(TOK)
    shards = []
    for c in range(N_CORES):
        ids_core = flat[c * TPC : (c + 1) * TPC]
        grid = ids_core.reshape(NCHUNK, P).T
        shards.append(np.ascontiguousarray(grid, dtype=np.int32))
    return shards


def kernel(x, W, b, trace=None):
    global LAST_EXEC_NS, LAST_RESULTS
    if trace is None:
        trace = bool(int(os.environ.get("EMB_TRACE", "0")))
    nc = _get_nc()
    x = np.ascontiguousarray(np.asarray(x, dtype=np.int32))
    Wf = np.ascontiguousarray(np.asarray(W, dtype=np.float32))
    bf = np.ascontiguousarray(np.asarray(b, dtype=np.float32)).reshape(D)
    id_shards = shard_ids(x)
    in_maps = [{"ids": id_shards[c], "W": Wf} for c in range(N_CORES)]
    res = run_bass_kernel_spmd(nc, in_maps, list(range(N_CORES)), trace=trace)
    LAST_EXEC_NS = res.exec_time_ns
    LAST_RESULTS = res
    outs = [res.results[c]["out"] for c in range(N_CORES)]
    full = np.concatenate(outs, axis=0)
    if np.any(bf):  # b is zero by spec; exact fallback if it ever weren't
        full = full + bf[None, :]
    return np.ascontiguousarray(full.reshape(B, S, D).astype(np.float32, copy=False))
